# revision 10
# baseline (speedup 1.0000x reference)
"""Trainium2 Bass kernel for a track-wise (ragged-sequence) attention layer.

Math (per track t of length L, per head h):
    qkv = values @ w_qkv.T + b_qkv                      # [N, 3*256]
    S   = q k^T / sqrt(Dh);  P = softmax(S);  ctx = P v
    out = ctx @ w_lin.T + b_lin

Device strategy (data parallel over tracks, 8 cores, no cross-core comm):
  - scores are computed directly transposed per head: ST = K @ Q^T, so
    exp(ST) is exactly the lhs^T the ctx matmul needs -- no PE transposes.
  - no max-subtraction in softmax (scores are ~N(0,1); exp is safe in fp32).
  - the softmax denominator is folded into the ctx matmul: lhsT = [V_h | 1],
    so one PE pass yields [ctx^T; rowsum] in PSUM.
  - RECIPROCAL DANCE (v3): the 1024 per-iter rowsums are extracted to SBUF
    by one DVE copy (together with ctx^T), DMA-reshaped [1,1024]->[128,8],
    ln+exp'd on the scalar engine at 8 elem/lane (~0.2us instead of the
    2.2us a [32,4,256]-layout reciprocal costs), DMA'd to a per-iter DRAM
    slot and partition-broadcast back to [32,4,256].  The normalize multiply
    runs on the otherwise-idle GPSIMD/Pool engine (SBUF x SBUF).
  - bf16 for et4/v_aug/ctx^T/cacc/rcp4/w_lin: same 1 cyc/row PE rate,
    halves SBUF, enables the deeper (lag-5) software pipeline.
  - QK bias adds are split between DVE tensor_scalar and scalar-engine
    activation(Identity, bias=per-partition) to balance the two drains
    (Identity/Copy share the ln/exp activation table set -- no reloads).
  - 1/sqrt(Dh) is folded into w_q/b_q on the host; b_v is folded into the
    final bias (softmax rows sum to 1): b_final = w_lin @ b_v + b_lin.
  - fp32 matmuls run as float32r (full rate with >=256 free); attention
    core matmuls run bf16.
"""

import os
import sys

import numpy as np

for _p in ("/opt/trn_rl_repo", "/root/.axon_site/_ro/trn_rl_repo"):
    if os.path.isdir(_p) and _p not in sys.path:
        sys.path.append(_p)

import ml_dtypes

import concourse.bass as bass
import concourse.tile as tile
from concourse import mybir
from concourse.bass_utils import run_bass_kernel_spmd

F32 = mybir.dt.float32
F32R = mybir.dt.float32r
BF16 = mybir.dt.bfloat16
EXP = mybir.ActivationFunctionType.Exp
LOG = mybir.ActivationFunctionType.Ln
IDENT = mybir.ActivationFunctionType.Identity
MUL = mybir.AluOpType.mult
ADD = mybir.AluOpType.add

N_CORES = 8
N, DIN, DOUT, H, T, L = 65536, 256, 256, 8, 256, 256

DH = DOUT // H          # 32
PC = N // N_CORES       # 8192 points per core
TPC = T // N_CORES      # 32 tracks per core
TPG = 4                 # tracks per group
NG = TPC // TPG         # 8 groups
GP = TPG * L            # 1024 points per group
MC_G = GP // 128        # 8 128-point chunks per group
NIT = NG * TPG * 2      # 64 iterations total

# tuning knobs
MULT_LAG = int(os.environ.get("MULT_LAG", "5"))
A1_SCALAR = int(os.environ.get("A1_SCALAR", "2"))   # of 4 rr units on scalar
DANCE_SYNC = os.environ.get("DANCE_SYNC", "1") == "1"  # dance DMAs on SP


class _TileContext(tile.TileContext):
    """TileContext whose final drain carries at most one semaphore wait per
    instruction (the walrus build in this container rejects multi-wait
    TPB_CTRL instructions)."""

    def _drain_and_barrier(self, tick_clock, wait_clock):
        super()._drain_and_barrier(tick_clock, wait_clock)
        self._split_multi_waits()

    def _split_multi_waits(self):
        nc = self.nc
        for f in nc.m.functions:
            for bb in f.blocks:
                changed = False
                new_insts = []
                for inst in bb.instructions:
                    si = inst.sync_info
                    if si is not None and len(si.on_wait) > 1:
                        waits = list(si.on_wait)
                        for w in waits[:-1]:
                            nop = mybir.InstNoOp(
                                name=f"I-{nc.next_id()}", ins=[], outs=[]
                            )
                            nop.engine = inst.engine
                            nop.sync_info = mybir.SyncInfo(
                                on_wait=[w], on_update=[]
                            )
                            new_insts.append(nop)
                        inst.sync_info = mybir.SyncInfo(
                            on_wait=[waits[-1]], on_update=list(si.on_update)
                        )
                        changed = True
                    new_insts.append(inst)
                if changed:
                    bb.instructions = new_insts


def _ensure_ntff_hook():
    """The agent image's ``antenv`` lacks ``axon_hooks``; provide it so
    ``run_bass_kernel_spmd(trace=True)`` can register the axon NTFF hook."""
    try:
        import antenv.axon_hooks  # noqa: F401
        return
    except ImportError:
        pass
    import types

    import antenv

    mod = types.ModuleType("antenv.axon_hooks")
    _hook = [None]
    mod.set_axon_ntff_profile_hook = lambda h: _hook.__setitem__(0, h)
    mod.get_axon_ntff_profile_hook = lambda: _hook[0]
    sys.modules["antenv.axon_hooks"] = mod
    antenv.axon_hooks = mod
    try:
        from trn_agent_boot.trn_boot import _ntff_profile_via_ctypes

        mod.set_axon_ntff_profile_hook(
            _ntff_profile_via_ctypes("/opt/axon/libaxon_pjrt.so")
        )
    except Exception as e:  # pragma: no cover - tracing is best-effort
        print(f"ntff hook setup failed: {e}", file=sys.stderr)


def _build_program(masked: bool):
    nc = bass.Bass("TRN2", target_bir_lowering=False, debug=False,
                   num_devices=N_CORES)

    xT = nc.dram_tensor("xT", [2, 128, PC], F32R, kind="ExternalInput").ap()
    wqk = nc.dram_tensor("wqk", [2, 128, 512], F32R, kind="ExternalInput").ap()
    wv = nc.dram_tensor("wv", [2, 128, 256], F32R, kind="ExternalInput").ap()
    wl = nc.dram_tensor("wl", [2, 128, 256], BF16, kind="ExternalInput").ap()
    bqk = nc.dram_tensor("bqk", [128, 4], F32, kind="ExternalInput").ap()
    bfin = nc.dram_tensor("bfin", [256], F32, kind="ExternalInput").ap()
    rdz = nc.dram_tensor("rdz", [NIT, 1024], BF16, kind="Internal").ap()
    if masked:
        mkf = nc.dram_tensor("mkf", [PC], F32, kind="ExternalInput").ap()
        mkp = nc.dram_tensor("mkp", [128, PC // 128], F32,
                             kind="ExternalInput").ap()
    out = nc.dram_tensor("out", [PC, DOUT], F32, kind="ExternalOutput").ap()

    dq = nc.sync if DANCE_SYNC else nc.gpsimd

    with _TileContext(nc) as tc:
        with (
            tc.tile_pool(name="consts", bufs=1) as consts,
            tc.tile_pool(name="xg", bufs=3) as xg_pool,
            tc.tile_pool(name="qk", bufs=2) as qk_pool,
            tc.tile_pool(name="vsb", bufs=2) as v_pool,
            tc.tile_pool(name="et", bufs=4) as et_pool,
            tc.tile_pool(name="ctxT", bufs=3) as ctx_pool,
            tc.tile_pool(name="osb", bufs=2) as o_pool,
            tc.tile_pool(name="cacc", bufs=MULT_LAG + 2) as cacc_pool,
            tc.tile_pool(name="zd", bufs=3) as z_pool,
            tc.tile_pool(name="rcp4", bufs=4) as rcp4_pool,
            tc.tile_pool(name="ps_st", bufs=2, space="PSUM") as ps_st,
            tc.tile_pool(name="ps_c4", bufs=2, space="PSUM") as ps_c4,
        ):
            # ---- constants -------------------------------------------------
            # only wqk/bqk gate the first projection; defer the other
            # const loads until after group 0's input is on the queue
            wqk_sb = consts.tile([128, 2, 512], F32R)
            nc.gpsimd.dma_start(out=wqk_sb, in_=wqk.rearrange("k p r -> p k r"))
            bqk_sb = consts.tile([128, 4], F32)
            nc.gpsimd.dma_start(out=bqk_sb, in_=bqk)
            wv_sb = consts.tile([128, 2, 256], F32R)
            wl_sb = consts.tile([128, 2, 256], BF16)
            bfin_sb = consts.tile([128, 256], F32)

            def load_late_consts():
                nc.gpsimd.dma_start(out=wv_sb,
                                    in_=wv.rearrange("k p r -> p k r"))
                nc.gpsimd.dma_start(out=wl_sb,
                                    in_=wl.rearrange("k p r -> p k r"))
                nc.gpsimd.dma_start(out=bfin_sb,
                                    in_=bfin.partition_broadcast(128))
            if masked:
                mkp_sb = consts.tile([128, PC // 128], F32)
                nc.gpsimd.dma_start(out=mkp_sb, in_=mkp)

            def emit_load(g):
                gsl = slice(g * GP, (g + 1) * GP)
                xg = xg_pool.tile([128, 2, GP], F32R, tag="xg", name=f"xg{g}")
                # two half-loads: the first a1 matmuls depend only on the
                # first 512 points, so they start as soon as half the group
                # lands (matters for group 0's prologue)
                half = GP // 2
                for hf in range(2):
                    hsl = slice(g * GP + hf * half, g * GP + (hf + 1) * half)
                    nc.gpsimd.dma_start(
                        out=xg[:, :, hf * half:(hf + 1) * half],
                        in_=xT[:, :, hsl].rearrange("k p n -> p k n"),
                    )
                mk_sb = None
                if masked:
                    mk_sb = et_pool.tile([128, GP], F32, tag="mk",
                                         name=f"mk{g}", bufs=2)
                    nc.gpsimd.dma_start(
                        out=mk_sb, in_=mkf[gsl].partition_broadcast(128)
                    )
                return xg, mk_sb

            def emit_a_units(g, xg, mk_sb):
                """Return (qk_sb, v_aug, units): units are small emission
                thunks (2 MMs + 1 DVE/scalar op each) to weave between B
                iterations so the PE stream stays dense across groups."""
                qk_sb = qk_pool.tile([128, 4, GP], F32R, tag="qk",
                                     name=f"qk{g}")
                v_aug = v_pool.tile([128, MC_G, H, 64], BF16, tag="va",
                                    name=f"va{g}")
                units = []

                def a1_unit(rr):
                    def _go():
                        ps = ps_st.tile([128, 2, 512], F32, tag="st",
                                        name=f"psa{g}_{rr}")
                        for n2 in range(2):
                            for k in range(2):
                                nc.tensor.matmul(
                                    ps[:, n2, :],
                                    wqk_sb[:, k, rr * 128:(rr + 1) * 128],
                                    xg[:, k, n2 * 512:(n2 + 1) * 512],
                                    start=(k == 0), stop=(k == 1),
                                )
                        if rr < 4 - A1_SCALAR:
                            nc.vector.tensor_scalar_add(
                                qk_sb[:, rr, :], ps, bqk_sb[:, rr:rr + 1],
                            )
                        else:
                            # scalar-engine bias add (Identity shares the
                            # ln/exp act table set: no table reload)
                            nc.scalar.activation(
                                qk_sb[:, rr, :], ps, IDENT,
                                bias=bqk_sb[:, rr:rr + 1],
                            )
                    return _go

                def mask_unit(rr):
                    def _go():
                        nc.vector.tensor_tensor(
                            qk_sb[:, rr, :], qk_sb[:, rr, :], mk_sb, MUL
                        )
                    return _go

                def memset_unit():
                    def _go():
                        nc.gpsimd.memset(v_aug[:, :, :, 32:33], 1.0)
                    return _go

                def a2_unit(mc0):
                    def _go():
                        ps = ps_st.tile([128, 2, 256], F32, tag="st",
                                        name=f"psv{g}_{mc0}")
                        for d in range(2):
                            mc = mc0 + d
                            for k in range(2):
                                nc.tensor.matmul(
                                    ps[:, d, :],
                                    xg[:, k, mc * 128:(mc + 1) * 128],
                                    wv_sb[:, k, :],
                                    start=(k == 0), stop=(k == 1),
                                )
                        for d in range(2):
                            mc = mc0 + d
                            if masked:
                                nc.vector.tensor_scalar_mul(
                                    v_aug[:, mc, :, 0:32],
                                    ps[:, d, :].rearrange(
                                        "p (h d2) -> p h d2", h=H),
                                    mkp_sb[:, g * MC_G + mc:
                                           g * MC_G + mc + 1],
                                )
                            else:
                                nc.vector.tensor_copy(
                                    v_aug[:, mc, :, 0:32],
                                    ps[:, d, :].rearrange(
                                        "p (h d2) -> p h d2", h=H),
                                )
                    return _go

                units.append(memset_unit())
                for rr in range(4):
                    units.append(a1_unit(rr))
                if masked:
                    units.append(mask_unit(2))
                    units.append(mask_unit(3))
                for mc0 in range(0, MC_G, 2):
                    units.append(a2_unit(mc0))
                return qk_sb, v_aug, units

            def emit_st_exp(g, qk_sb, t, hg, j_global):
                # ST = K@Q^T then exp, two 2-head sub-batches
                tsl = slice(t * 256, (t + 1) * 256)
                et4 = et_pool.tile([128, 4, 512], BF16, tag="et",
                                   name=f"et{j_global}")
                st2s = [ps_st.tile([128, 2, 512], F32, tag="st",
                                   name=f"st{j_global}_{sb}")
                        for sb in range(2)]
                for j in range(2):
                    for hh in range(4):
                        po = hh * 32
                        nc.tensor.matmul(
                            st2s[hh // 2][:, hh % 2, j * 256:(j + 1) * 256],
                            qk_sb[po:po + 32, 2 + hg,
                                  t * 256 + j * 128:t * 256 + (j + 1) * 128],
                            qk_sb[po:po + 32, hg, tsl],
                            start=True, stop=True,
                            tile_position=(po, 0),
                        )
                for sb in range(2):
                    nc.scalar.activation(
                        et4[:, sb * 2:(sb + 1) * 2, :], st2s[sb], EXP
                    )
                return et4

            def emit_ctx(g, v_aug, t, hg, et4, j_global):
                # [ctx^T ; rowsum] accumulate: lhsT = [V_h | 1]
                acc4 = ps_c4.tile([33, 4, 256], F32, tag="c4",
                                  name=f"acc{j_global}")
                for hh in range(4):
                    h = hg * 4 + hh
                    for j in range(2):
                        nc.tensor.matmul(
                            acc4[:, hh, :],
                            v_aug[:, t * 2 + j, h, 0:33],
                            et4[:, hh, j * 256:(j + 1) * 256],
                            start=(j == 0), stop=(j == 1),
                        )
                return acc4

            # ---- reciprocal dance stages ----------------------------------
            state = {}   # j_global -> dict with tiles + (g, t, hg)

            def emit_copy_t1(j):
                st = state[j]
                cacc = cacc_pool.tile([33, 4, 256], BF16, tag="cacc",
                                      name=f"cacc{j}")
                nc.vector.tensor_copy(cacc, st["acc4"])
                st["cacc"] = cacc
                st["acc4"] = None
                zrow = z_pool.tile([128, 8], BF16, tag="zrow",
                                   name=f"zrow{j}")
                # SP queue holds only T1/T3 dance triggers, so its blocking
                # waits never stall a compute engine's stream
                nc.sync.dma_start(out=zrow, in_=cacc[32:33, :, :])
                st["zrow"] = zrow

            def emit_recip_t2(j):
                st = state[j]
                zl = z_pool.tile([128, 8], F32, tag="zl", name=f"zl{j}")
                zrcp = z_pool.tile([128, 8], BF16, tag="zrcp",
                                   name=f"zrcp{j}")
                nc.scalar.activation(zl, st["zrow"], LOG)
                nc.scalar.activation(zrcp, zl, EXP, scale=-1.0)
                st["zrow"] = None
                # trigger from the Act queue right after the exp: no wait
                nc.scalar.dma_start(out=rdz[j, :], in_=zrcp)
                st["zrcp"] = zrcp

            def emit_t3(j):
                st = state[j]
                rcp4 = rcp4_pool.tile([32, 4, 256], BF16, tag="rcp4",
                                      name=f"rcp4_{j}")
                nc.sync.dma_start(out=rcp4,
                                  in_=rdz[j, :].partition_broadcast(32))
                st["rcp4"] = rcp4

            def emit_mult(j, ctxT_tiles):
                st = state.pop(j)
                g, t, hg = st["g"], st["t"], st["hg"]
                tsl = slice(t * 256, (t + 1) * 256)
                ctxT_sb = ctxT_tiles[g]
                for hh in range(4):
                    nc.gpsimd.tensor_tensor(
                        ctxT_sb[hh * 32:(hh + 1) * 32, hg, tsl],
                        st["cacc"][0:32, hh, :],
                        st["rcp4"][:, hh, :], MUL,
                    )

            def dance_pre(k, ctxT_tiles):
                """Stages for older iterations, run at iteration-slot k."""
                if k - MULT_LAG in state:
                    emit_mult(k - MULT_LAG, ctxT_tiles)
                if k - 3 in state and "rcp4" not in state[k - 3]:
                    emit_t3(k - 3)
                if k - 2 in state and "zrcp" not in state[k - 2]:
                    emit_recip_t2(k - 2)

            def emit_c_units(g, ctxT_sb):
                o_sb = o_pool.tile([128, MC_G, 256], F32, tag="o",
                                   name=f"o{g}")

                def c_unit(mc0):
                    def _go():
                        ps = ps_st.tile([128, 2, 256], F32, tag="st",
                                        name=f"psc{g}_{mc0}")
                        for d in range(2):
                            mc = mc0 + d
                            for kc in range(2):
                                nc.tensor.matmul(
                                    ps[:, d, :],
                                    ctxT_sb[:, kc, mc * 128:(mc + 1) * 128],
                                    wl_sb[:, kc, :],
                                    start=(kc == 0), stop=(kc == 1),
                                )
                        for d in range(2):
                            mc = mc0 + d
                            nc.vector.tensor_tensor(
                                o_sb[:, mc, :], ps[:, d, :], bfin_sb, ADD
                            )
                        # ship this 256-point slice immediately: overlaps
                        # the store with remaining compute, shrinking the tail
                        rsl = slice(g * GP + mc0 * 128,
                                    g * GP + (mc0 + 2) * 128)
                        nc.gpsimd.dma_start(
                            out=out[rsl, :].rearrange(
                                "(m p) n -> p m n", p=128),
                            in_=o_sb[:, mc0:mc0 + 2, :],
                        )
                    return _go

                return [c_unit(mc0) for mc0 in range(0, MC_G, 2)]

            def weave(c_units, a_units):
                """Interleave so c_unit[ci] (reads ctxT pts of track ci of
                the PREVIOUS group, whose norm mults land at slot
                2*ci+1+MULT_LAG-8 of this group) sits at a weave position
                past its data dependency (+1 slot of margin)."""
                total = len(c_units) + len(a_units)
                units = []
                ai = ci = 0
                for s in range(total):
                    thr = None
                    if ci < len(c_units):
                        thr = total * (2 * ci + MULT_LAG - 6) / 8.0
                    if thr is not None and (s >= thr or ai >= len(a_units)):
                        units.append(c_units[ci])
                        ci += 1
                    else:
                        units.append(a_units[ai])
                        ai += 1
                return units

            # ---- software-pipelined schedule ------------------------------
            xg0 = emit_load(0)
            load_late_consts()
            qk0, va0, units0 = emit_a_units(0, *xg0)
            for u in units0:
                u()
            ab = {0: (qk0, va0)}
            ctxT_tiles = {}
            c_carry = []
            iters = [(t, hg) for t in range(TPG) for hg in (0, 1)]
            for g in range(NG):
                qk_sb, v_aug = ab.pop(g)
                ctxT_tiles[g] = ctx_pool.tile([128, 2, GP], BF16, tag="ctxT",
                                              name=f"ctxT{g}")
                a_units = []
                if g + 1 < NG:
                    xgn = emit_load(g + 1)
                    qkn, van, aun = emit_a_units(g + 1, *xgn)
                    ab[g + 1] = (qkn, van)
                    a_units = aun
                units = weave(list(c_carry), a_units)
                c_carry = []
                ui = 0
                ctx_pend = None
                for i, (t, hg) in enumerate(iters):
                    k = g * 8 + i
                    dance_pre(k, ctxT_tiles)
                    et4 = emit_st_exp(g, qk_sb, t, hg, k)
                    if ctx_pend is not None:
                        pk, pt, phg, pet = ctx_pend
                        acc4 = emit_ctx(g, v_aug, pt, phg, pet, pk)
                        state[pk] = dict(acc4=acc4, g=g, t=pt, hg=phg)
                        emit_copy_t1(pk)
                    ctx_pend = (k, t, hg, et4)
                    take = (len(units) * (i + 1)) // len(iters) - ui
                    for _ in range(take):
                        units[ui]()
                        ui += 1
                while ui < len(units):
                    units[ui]()
                    ui += 1
                # group-final ctx (iter k=g*8+7): emit now, copy next slot
                pk, pt, phg, pet = ctx_pend
                acc4 = emit_ctx(g, v_aug, pt, phg, pet, pk)
                state[pk] = dict(acc4=acc4, g=g, t=pt, hg=phg)
                emit_copy_t1(pk)
                c_carry = emit_c_units(g, ctxT_tiles[g])

            # ---- epilogue: drain the dance, last group's C phase ----------
            k = NG * 8
            while state:
                dance_pre(k, ctxT_tiles)
                k += 1
            for u in c_carry:
                u()

    return nc


_PROG_CACHE = {}


def _get_program(masked: bool):
    if masked not in _PROG_CACHE:
        _PROG_CACHE[masked] = _build_program(masked)
    return _PROG_CACHE[masked]


def _prep_host(values, w_qkv, b_qkv, w_lin, b_lin):
    """Host-side weight preprocessing (all cheap, shared across cores)."""
    scale = 1.0 / np.sqrt(DH)
    w_qkv = np.asarray(w_qkv, np.float32).copy()
    b_qkv = np.asarray(b_qkv, np.float32).copy()
    w_lin = np.asarray(w_lin, np.float32)
    b_lin = np.asarray(b_lin, np.float32)
    w_qkv[:DOUT] *= scale
    b_qkv[:DOUT] *= scale

    wqk = np.ascontiguousarray(
        w_qkv[:2 * DOUT].T.reshape(2, 128, 512)
    )  # [k-chunk, k-part, row]
    wv = np.ascontiguousarray(w_qkv[2 * DOUT:].T.reshape(2, 128, 256))
    wl = np.ascontiguousarray(
        w_lin.T.reshape(2, 128, 256)).astype(ml_dtypes.bfloat16)
    bqk = np.ascontiguousarray(b_qkv[:2 * DOUT].reshape(4, 128).T)
    b_v = b_qkv[2 * DOUT:]  # unscaled: only the q section was scaled above
    bfin = (w_lin @ b_v + b_lin).astype(np.float32)
    return wqk, wv, wl, bqk, bfin


def _run(values_padded, mask, w_arrs, trace=False):
    """values_padded: [N, 256] in track-padded order; mask: None or [N]."""
    wqk, wv, wl, bqk, bfin = w_arrs
    masked = mask is not None
    nc = _get_program(masked)

    in_maps = []
    for c in range(N_CORES):
        sl = slice(c * PC, (c + 1) * PC)
        xTc = np.ascontiguousarray(
            values_padded[sl].T.reshape(2, 128, PC)
        )
        m = dict(xT=xTc, wqk=wqk, wv=wv, wl=wl, bqk=bqk, bfin=bfin)
        if masked:
            mc_ = np.ascontiguousarray(mask[sl], np.float32)
            m["mkf"] = mc_
            m["mkp"] = np.ascontiguousarray(mc_.reshape(PC // 128, 128).T)
        in_maps.append(m)

    if trace:
        _ensure_ntff_hook()
    res = run_bass_kernel_spmd(nc, in_maps, list(range(N_CORES)), trace=trace)
    outp = np.concatenate([res.results[c]["out"] for c in range(N_CORES)], 0)
    return outp, res


LAST_RESULTS = None


def kernel(values, w_qkv, b_qkv, w_lin, b_lin, track_ids, n_tracks,
           num_heads, _trace=False):
    global LAST_RESULTS
    values = np.asarray(values, np.float32)
    track_ids = np.asarray(track_ids, np.int32)
    n_tracks_i = int(n_tracks)
    num_heads_i = int(num_heads)
    assert values.shape == (N, DIN) and n_tracks_i == T and num_heads_i == H, (
        "kernel compiled for N=65536, d=256, T=256, H=8"
    )

    w_arrs = _prep_host(values, w_qkv, b_qkv, w_lin, b_lin)

    counts = np.bincount(track_ids, minlength=T)
    equal = bool((counts == L).all())

    if equal:
        outp, res = _run(values, None, w_arrs, trace=_trace)
        LAST_RESULTS = res
        return outp

    # general sorted-ragged path: scatter to padded [T, L] grid on host,
    # run the same device kernel with padding masked out of K and V, then
    # gather back (mirroring jax's oob-drop scatter / clip gather).
    starts = np.concatenate([[0], np.cumsum(counts)[:-1]])
    pos = np.arange(N, dtype=np.int64) - starts[track_ids]
    keep = pos < L
    rows = track_ids.astype(np.int64) * L + np.minimum(pos, L - 1)
    padded = np.zeros((T * L, DIN), np.float32)
    padded[rows[keep]] = values[keep]
    mask = np.zeros(T * L, np.float32)
    mask[rows[keep]] = 1.0
    outp, res = _run(padded, mask, w_arrs, trace=_trace)
    LAST_RESULTS = res
    return np.ascontiguousarray(outp[rows])


# revision 18
# speedup vs baseline: 1.1341x; 1.1341x over previous
"""Trainium2 Bass kernel for a track-wise (ragged-sequence) attention layer.

Math (per track t of length L, per head h):
    qkv = values @ w_qkv.T + b_qkv                      # [N, 3*256]
    S   = q k^T / sqrt(Dh);  P = softmax(S);  ctx = P v
    out = ctx @ w_lin.T + b_lin

Device strategy (data parallel over tracks, 8 cores, no cross-core comm):
  - scores are computed directly transposed per head: ST = K @ Q^T, so
    exp(ST) is exactly the lhs^T the ctx matmul needs -- no PE transposes.
  - no max-subtraction in softmax (scores are ~N(0,1); exp is safe in fp32).
  - the softmax denominator is folded into the ctx matmul: lhsT = [V_h | 1],
    so one PE pass yields [ctx^T; rowsum] in PSUM.
  - RECIPROCAL DANCE (v3): the 1024 per-iter rowsums are extracted to SBUF
    by one DVE copy (together with ctx^T), DMA-reshaped [1,1024]->[128,8],
    ln+exp'd on the scalar engine at 8 elem/lane (~0.2us instead of the
    2.2us a [32,4,256]-layout reciprocal costs), DMA'd to a per-iter DRAM
    slot and partition-broadcast back to [32,4,256].  The normalize multiply
    runs on the otherwise-idle GPSIMD/Pool engine (SBUF x SBUF).
  - bf16 for et4/v_aug/ctx^T/cacc/rcp4/w_lin: same 1 cyc/row PE rate,
    halves SBUF, enables the deeper (lag-5) software pipeline.
  - QK bias adds are split between DVE tensor_scalar and scalar-engine
    activation(Identity, bias=per-partition) to balance the two drains
    (Identity/Copy share the ln/exp activation table set -- no reloads).
  - 1/sqrt(Dh) is folded into w_q/b_q on the host; b_v is folded into the
    final bias (softmax rows sum to 1): b_final = w_lin @ b_v + b_lin.
  - fp32 matmuls run as float32r (full rate with >=256 free); attention
    core matmuls run bf16.
"""

import os
import sys

import numpy as np

for _p in ("/opt/trn_rl_repo", "/root/.axon_site/_ro/trn_rl_repo"):
    if os.path.isdir(_p) and _p not in sys.path:
        sys.path.append(_p)

import ml_dtypes

import concourse.bass as bass
import concourse.tile as tile
from concourse import mybir
from concourse.bass_utils import run_bass_kernel_spmd

F32 = mybir.dt.float32
F32R = mybir.dt.float32r
BF16 = mybir.dt.bfloat16
EXP = mybir.ActivationFunctionType.Exp
LOG = mybir.ActivationFunctionType.Ln
IDENT = mybir.ActivationFunctionType.Identity
MUL = mybir.AluOpType.mult
ADD = mybir.AluOpType.add

N_CORES = 8
N, DIN, DOUT, H, T, L = 65536, 256, 256, 8, 256, 256

DH = DOUT // H          # 32
PC = N // N_CORES       # 8192 points per core
TPC = T // N_CORES      # 32 tracks per core
TPG = 4                 # tracks per group
NG = TPC // TPG         # 8 groups
GP = TPG * L            # 1024 points per group
MC_G = GP // 128        # 8 128-point chunks per group
NIT = NG * TPG * 2      # 64 iterations total

# tuning knobs
MULT_LAG = int(os.environ.get("MULT_LAG", "5"))
A1_SCALAR = int(os.environ.get("A1_SCALAR", "0"))   # of 4 rr units on scalar
DANCE_SYNC = os.environ.get("DANCE_SYNC", "1") == "1"  # dance DMAs on SP


class _TileContext(tile.TileContext):
    """TileContext whose final drain carries at most one semaphore wait per
    instruction (the walrus build in this container rejects multi-wait
    TPB_CTRL instructions)."""

    def _drain_and_barrier(self, tick_clock, wait_clock):
        super()._drain_and_barrier(tick_clock, wait_clock)
        self._split_multi_waits()

    def _split_multi_waits(self):
        nc = self.nc
        for f in nc.m.functions:
            for bb in f.blocks:
                changed = False
                new_insts = []
                for inst in bb.instructions:
                    si = inst.sync_info
                    if si is not None and len(si.on_wait) > 1:
                        waits = list(si.on_wait)
                        for w in waits[:-1]:
                            nop = mybir.InstNoOp(
                                name=f"I-{nc.next_id()}", ins=[], outs=[]
                            )
                            nop.engine = inst.engine
                            nop.sync_info = mybir.SyncInfo(
                                on_wait=[w], on_update=[]
                            )
                            new_insts.append(nop)
                        inst.sync_info = mybir.SyncInfo(
                            on_wait=[waits[-1]], on_update=list(si.on_update)
                        )
                        changed = True
                    new_insts.append(inst)
                if changed:
                    bb.instructions = new_insts


def _ensure_ntff_hook():
    """The agent image's ``antenv`` lacks ``axon_hooks``; provide it so
    ``run_bass_kernel_spmd(trace=True)`` can register the axon NTFF hook."""
    try:
        import antenv.axon_hooks  # noqa: F401
        return
    except ImportError:
        pass
    import types

    import antenv

    mod = types.ModuleType("antenv.axon_hooks")
    _hook = [None]
    mod.set_axon_ntff_profile_hook = lambda h: _hook.__setitem__(0, h)
    mod.get_axon_ntff_profile_hook = lambda: _hook[0]
    sys.modules["antenv.axon_hooks"] = mod
    antenv.axon_hooks = mod
    try:
        from trn_agent_boot.trn_boot import _ntff_profile_via_ctypes

        mod.set_axon_ntff_profile_hook(
            _ntff_profile_via_ctypes("/opt/axon/libaxon_pjrt.so")
        )
    except Exception as e:  # pragma: no cover - tracing is best-effort
        print(f"ntff hook setup failed: {e}", file=sys.stderr)


def _build_program(masked: bool):
    nc = bass.Bass("TRN2", target_bir_lowering=False, debug=False,
                   num_devices=N_CORES)

    xT = nc.dram_tensor("xT", [2, 128, PC], F32R, kind="ExternalInput").ap()
    wqk = nc.dram_tensor("wqk", [2, 128, 512], F32R, kind="ExternalInput").ap()
    wv = nc.dram_tensor("wv", [2, 128, 256], F32R, kind="ExternalInput").ap()
    wl = nc.dram_tensor("wl", [2, 128, 256], BF16, kind="ExternalInput").ap()
    bqk = nc.dram_tensor("bqk", [128, 4], F32, kind="ExternalInput").ap()
    bfin = nc.dram_tensor("bfin", [256], F32, kind="ExternalInput").ap()
    rdz = nc.dram_tensor("rdz", [NIT, 1024], BF16, kind="Internal").ap()
    if masked:
        mkf = nc.dram_tensor("mkf", [PC], F32, kind="ExternalInput").ap()
        mkp = nc.dram_tensor("mkp", [128, PC // 128], F32,
                             kind="ExternalInput").ap()
    out = nc.dram_tensor("out", [PC, DOUT], F32, kind="ExternalOutput").ap()

    dq = nc.sync if DANCE_SYNC else nc.gpsimd

    with _TileContext(nc) as tc:
        with (
            tc.tile_pool(name="consts", bufs=1) as consts,
            tc.tile_pool(name="xg", bufs=3) as xg_pool,
            tc.tile_pool(name="qk", bufs=2) as qk_pool,
            tc.tile_pool(name="vsb", bufs=2) as v_pool,
            tc.tile_pool(name="et", bufs=4) as et_pool,
            tc.tile_pool(name="ctxT", bufs=3) as ctx_pool,
            tc.tile_pool(name="osb", bufs=2) as o_pool,
            tc.tile_pool(name="cacc", bufs=MULT_LAG + 2) as cacc_pool,
            tc.tile_pool(name="zd", bufs=3) as z_pool,
            tc.tile_pool(name="rcp4", bufs=4) as rcp4_pool,
            tc.tile_pool(name="ps_st", bufs=2, space="PSUM") as ps_st,
            tc.tile_pool(name="ps_acc", bufs=1, space="PSUM") as ps_acc,
            tc.tile_pool(name="ps_misc", bufs=2, space="PSUM") as ps_misc,
        ):
            # ---- constants -------------------------------------------------
            # only wqk/bqk gate the first projection; defer the other
            # const loads until after group 0's input is on the queue
            wqk_sb = consts.tile([128, 2, 512], F32R)
            nc.gpsimd.dma_start(out=wqk_sb, in_=wqk.rearrange("k p r -> p k r"))
            bqk_sb = consts.tile([128, 4], F32)
            nc.gpsimd.dma_start(out=bqk_sb, in_=bqk)
            wv_sb = consts.tile([128, 2, 256], F32R)
            wl_sb = consts.tile([128, 2, 256], BF16)
            bfin_sb = consts.tile([128, 256], F32)

            def load_late_consts():
                nc.gpsimd.dma_start(out=wv_sb,
                                    in_=wv.rearrange("k p r -> p k r"))
                nc.gpsimd.dma_start(out=wl_sb,
                                    in_=wl.rearrange("k p r -> p k r"))
                nc.gpsimd.dma_start(out=bfin_sb,
                                    in_=bfin.partition_broadcast(128))
            if masked:
                mkp_sb = consts.tile([128, PC // 128], F32)
                nc.gpsimd.dma_start(out=mkp_sb, in_=mkp)

            def emit_load(g):
                gsl = slice(g * GP, (g + 1) * GP)
                xg = xg_pool.tile([128, 2, GP], F32R, tag="xg", name=f"xg{g}")
                # two half-loads: the first a1 matmuls depend only on the
                # first 512 points, so they start as soon as half the group
                # lands (matters for group 0's prologue)
                half = GP // 2
                for hf in range(2):
                    hsl = slice(g * GP + hf * half, g * GP + (hf + 1) * half)
                    nc.sync.dma_start(
                        out=xg[:, :, hf * half:(hf + 1) * half],
                        in_=xT[:, :, hsl].rearrange("k p n -> p k n"),
                    )
                mk_sb = None
                if masked:
                    mk_sb = et_pool.tile([128, GP], F32, tag="mk",
                                         name=f"mk{g}", bufs=2)
                    nc.gpsimd.dma_start(
                        out=mk_sb, in_=mkf[gsl].partition_broadcast(128)
                    )
                return xg, mk_sb

            def emit_a_units(g, xg, mk_sb):
                """Return (qk_sb, v_aug, units): units are small emission
                thunks (2 MMs + 1 DVE/scalar op each) to weave between B
                iterations so the PE stream stays dense across groups."""
                qk_sb = qk_pool.tile([128, 4, GP], F32R, tag="qk",
                                     name=f"qk{g}")
                v_aug = v_pool.tile([128, MC_G, H, 64], BF16, tag="va",
                                    name=f"va{g}")
                units = []

                def a1_unit(rr):
                    def _go():
                        # two 1-bank tiles so the misc pool stays 1-bank
                        pss = [ps_misc.tile([128, 512], F32, tag="ms",
                                            name=f"psa{g}_{rr}_{n2}")
                               for n2 in range(2)]
                        for n2 in range(2):
                            for k in range(2):
                                nc.tensor.matmul(
                                    pss[n2],
                                    wqk_sb[:, k, rr * 128:(rr + 1) * 128],
                                    xg[:, k, n2 * 512:(n2 + 1) * 512],
                                    start=(k == 0), stop=(k == 1),
                                )
                        for n2 in range(2):
                            if rr < 4 - A1_SCALAR:
                                nc.vector.tensor_scalar_add(
                                    qk_sb[:, rr, n2 * 512:(n2 + 1) * 512],
                                    pss[n2], bqk_sb[:, rr:rr + 1],
                                )
                            else:
                                # scalar-engine bias add (Identity shares
                                # the ln/exp act table set: no reload)
                                nc.scalar.activation(
                                    qk_sb[:, rr, n2 * 512:(n2 + 1) * 512],
                                    pss[n2], IDENT,
                                    bias=bqk_sb[:, rr:rr + 1],
                                )
                    return _go

                def mask_unit(rr):
                    def _go():
                        nc.vector.tensor_tensor(
                            qk_sb[:, rr, :], qk_sb[:, rr, :], mk_sb, MUL
                        )
                    return _go

                def memset_unit():
                    def _go():
                        nc.gpsimd.memset(v_aug[:, :, :, 32:33], 1.0)
                    return _go

                def a2_unit(mc0):
                    def _go():
                        ps = ps_misc.tile([128, 2, 256], F32, tag="ms",
                                          name=f"psv{g}_{mc0}")
                        for d in range(2):
                            mc = mc0 + d
                            for k in range(2):
                                nc.tensor.matmul(
                                    ps[:, d, :],
                                    xg[:, k, mc * 128:(mc + 1) * 128],
                                    wv_sb[:, k, :],
                                    start=(k == 0), stop=(k == 1),
                                )
                        for d in range(2):
                            mc = mc0 + d
                            if masked:
                                nc.vector.tensor_scalar_mul(
                                    v_aug[:, mc, :, 0:32],
                                    ps[:, d, :].rearrange(
                                        "p (h d2) -> p h d2", h=H),
                                    mkp_sb[:, g * MC_G + mc:
                                           g * MC_G + mc + 1],
                                )
                            else:
                                nc.vector.tensor_copy(
                                    v_aug[:, mc, :, 0:32],
                                    ps[:, d, :].rearrange(
                                        "p (h d2) -> p h d2", h=H),
                                )
                    return _go

                units.append(memset_unit())
                for rr in range(4):
                    units.append(a1_unit(rr))
                if masked:
                    units.append(mask_unit(2))
                    units.append(mask_unit(3))
                for mc0 in range(0, MC_G, 2):
                    units.append(a2_unit(mc0))
                return qk_sb, v_aug, units

            def emit_st_exp(g, qk_sb, t, hg, j_global):
                # ST = K@Q^T then exp, two 2-head sub-batches
                tsl = slice(t * 256, (t + 1) * 256)
                et4 = et_pool.tile([128, 4, 512], BF16, tag="et",
                                   name=f"et{j_global}")
                st2s = [ps_st.tile([128, 2, 512], F32, tag="st",
                                   name=f"st{j_global}_{sb}")
                        for sb in range(2)]
                for j in range(2):
                    for hh in range(4):
                        po = hh * 32
                        nc.tensor.matmul(
                            st2s[hh // 2][:, hh % 2, j * 256:(j + 1) * 256],
                            qk_sb[po:po + 32, 2 + hg,
                                  t * 256 + j * 128:t * 256 + (j + 1) * 128],
                            qk_sb[po:po + 32, hg, tsl],
                            start=True, stop=True,
                            tile_position=(po, 0),
                        )
                for sb in range(2):
                    nc.scalar.activation(
                        et4[:, sb * 2:(sb + 1) * 2, :], st2s[sb], EXP
                    )
                return et4

            def emit_ctx(g, v_aug, t, hg, et4, j_global):
                # [ctx^T ; rowsum] accumulate: lhsT = [V_h | 1]
                acc4 = ps_acc.tile([33, 4, 256], F32, tag="c4",
                                   name=f"acc{j_global}")
                for hh in range(4):
                    h = hg * 4 + hh
                    for j in range(2):
                        nc.tensor.matmul(
                            acc4[:, hh, :],
                            v_aug[:, t * 2 + j, h, 0:33],
                            et4[:, hh, j * 256:(j + 1) * 256],
                            start=(j == 0), stop=(j == 1),
                        )
                return acc4

            # ---- reciprocal dance stages ----------------------------------
            state = {}   # j_global -> dict with tiles + (g, t, hg)

            def emit_copy_t1(j):
                st = state[j]
                cacc = cacc_pool.tile([33, 4, 256], BF16, tag="cacc",
                                      name=f"cacc{j}")
                nc.vector.tensor_copy(cacc, st["acc4"])
                st["cacc"] = cacc
                st["acc4"] = None
                zrow = z_pool.tile([128, 8], BF16, tag="zrow",
                                   name=f"zrow{j}")
                # SP queue holds only T1/T3 dance triggers, so its blocking
                # waits never stall a compute engine's stream
                nc.sync.dma_start(out=zrow, in_=cacc[32:33, :, :])
                st["zrow"] = zrow

            def emit_recip_t2(j):
                st = state[j]
                zl = z_pool.tile([128, 8], F32, tag="zl", name=f"zl{j}")
                zrcp = z_pool.tile([128, 8], BF16, tag="zrcp",
                                   name=f"zrcp{j}")
                nc.scalar.activation(zl, st["zrow"], LOG)
                nc.scalar.activation(zrcp, zl, EXP, scale=-1.0)
                st["zrow"] = None
                # trigger from the Act queue right after the exp: no wait
                nc.scalar.dma_start(out=rdz[j, :], in_=zrcp)
                st["zrcp"] = zrcp

            def emit_t3(j):
                st = state[j]
                rcp4 = rcp4_pool.tile([32, 4, 256], BF16, tag="rcp4",
                                      name=f"rcp4_{j}")
                nc.sync.dma_start(out=rcp4,
                                  in_=rdz[j, :].partition_broadcast(32))
                st["rcp4"] = rcp4

            def emit_mult(j, ctxT_tiles):
                st = state.pop(j)
                g, t, hg = st["g"], st["t"], st["hg"]
                tsl = slice(t * 256, (t + 1) * 256)
                ctxT_sb = ctxT_tiles[g]
                for hh in range(4):
                    nc.gpsimd.tensor_tensor(
                        ctxT_sb[hh * 32:(hh + 1) * 32, hg, tsl],
                        st["cacc"][0:32, hh, :],
                        st["rcp4"][:, hh, :], MUL,
                    )

            def dance_pre(k, ctxT_tiles):
                """Stages for older iterations, run at iteration-slot k."""
                if k - MULT_LAG in state:
                    emit_mult(k - MULT_LAG, ctxT_tiles)
                if k - 3 in state and "rcp4" not in state[k - 3]:
                    emit_t3(k - 3)
                if k - 2 in state and "zrcp" not in state[k - 2]:
                    emit_recip_t2(k - 2)

            def emit_c_units(g, ctxT_sb):
                o_sb = o_pool.tile([128, MC_G, 256], F32, tag="o",
                                   name=f"o{g}")

                def c_unit(mc0):
                    def _go():
                        ps = ps_misc.tile([128, 2, 256], F32, tag="ms",
                                          name=f"psc{g}_{mc0}")
                        for d in range(2):
                            mc = mc0 + d
                            for kc in range(2):
                                nc.tensor.matmul(
                                    ps[:, d, :],
                                    ctxT_sb[:, kc, mc * 128:(mc + 1) * 128],
                                    wl_sb[:, kc, :],
                                    start=(kc == 0), stop=(kc == 1),
                                )
                        for d in range(2):
                            mc = mc0 + d
                            nc.vector.tensor_tensor(
                                o_sb[:, mc, :], ps[:, d, :], bfin_sb, ADD
                            )
                        # ship this 256-point slice immediately: overlaps
                        # the store with remaining compute, shrinking the tail
                        rsl = slice(g * GP + mc0 * 128,
                                    g * GP + (mc0 + 2) * 128)
                        nc.sync.dma_start(
                            out=out[rsl, :].rearrange(
                                "(m p) n -> p m n", p=128),
                            in_=o_sb[:, mc0:mc0 + 2, :],
                        )
                    return _go

                return [c_unit(mc0) for mc0 in range(0, MC_G, 2)]

            def weave(c_units, a_units):
                """Interleave so c_unit[ci] (reads ctxT pts of track ci of
                the PREVIOUS group, whose norm mults land at slot
                2*ci+1+MULT_LAG-8 of this group) sits at a weave position
                past its data dependency (+1 slot of margin)."""
                total = len(c_units) + len(a_units)
                units = []
                ai = ci = 0
                for s in range(total):
                    thr = None
                    if ci < len(c_units):
                        thr = total * (2 * ci + MULT_LAG - 6) / 8.0
                    if thr is not None and (s >= thr or ai >= len(a_units)):
                        units.append(c_units[ci])
                        ci += 1
                    else:
                        units.append(a_units[ai])
                        ai += 1
                return units

            # ---- software-pipelined schedule ------------------------------
            xg0 = emit_load(0)
            load_late_consts()
            qk0, va0, units0 = emit_a_units(0, *xg0)
            for u in units0:
                u()
            ab = {0: (qk0, va0)}
            ctxT_tiles = {}
            c_carry = []
            iters = [(t, hg) for t in range(TPG) for hg in (0, 1)]
            for g in range(NG):
                qk_sb, v_aug = ab.pop(g)
                ctxT_tiles[g] = ctx_pool.tile([128, 2, GP], BF16, tag="ctxT",
                                              name=f"ctxT{g}")
                a_units = []
                if g + 1 < NG:
                    xgn = emit_load(g + 1)
                    qkn, van, aun = emit_a_units(g + 1, *xgn)
                    ab[g + 1] = (qkn, van)
                    a_units = aun
                units = weave(list(c_carry), a_units)
                c_carry = []
                ui = 0
                ctx_pend = None
                for i, (t, hg) in enumerate(iters):
                    k = g * 8 + i
                    dance_pre(k, ctxT_tiles)
                    et4 = emit_st_exp(g, qk_sb, t, hg, k)
                    if ctx_pend is not None:
                        pk, pt, phg, pet = ctx_pend
                        acc4 = emit_ctx(g, v_aug, pt, phg, pet, pk)
                        state[pk] = dict(acc4=acc4, g=g, t=pt, hg=phg)
                        emit_copy_t1(pk)
                    ctx_pend = (k, t, hg, et4)
                    take = (len(units) * (i + 1)) // len(iters) - ui
                    for _ in range(take):
                        units[ui]()
                        ui += 1
                while ui < len(units):
                    units[ui]()
                    ui += 1
                # group-final ctx (iter k=g*8+7): emit now, copy next slot
                pk, pt, phg, pet = ctx_pend
                acc4 = emit_ctx(g, v_aug, pt, phg, pet, pk)
                state[pk] = dict(acc4=acc4, g=g, t=pt, hg=phg)
                emit_copy_t1(pk)
                c_carry = emit_c_units(g, ctxT_tiles[g])

            # ---- epilogue: drain the dance, last group's C phase ----------
            k = NG * 8
            while state:
                dance_pre(k, ctxT_tiles)
                k += 1
            for u in c_carry:
                u()

    return nc


_PROG_CACHE = {}


def _get_program(masked: bool):
    if masked not in _PROG_CACHE:
        _PROG_CACHE[masked] = _build_program(masked)
    return _PROG_CACHE[masked]


def _prep_host(values, w_qkv, b_qkv, w_lin, b_lin):
    """Host-side weight preprocessing (all cheap, shared across cores)."""
    scale = 1.0 / np.sqrt(DH)
    w_qkv = np.asarray(w_qkv, np.float32).copy()
    b_qkv = np.asarray(b_qkv, np.float32).copy()
    w_lin = np.asarray(w_lin, np.float32)
    b_lin = np.asarray(b_lin, np.float32)
    w_qkv[:DOUT] *= scale
    b_qkv[:DOUT] *= scale

    wqk = np.ascontiguousarray(
        w_qkv[:2 * DOUT].T.reshape(2, 128, 512)
    )  # [k-chunk, k-part, row]
    wv = np.ascontiguousarray(w_qkv[2 * DOUT:].T.reshape(2, 128, 256))
    wl = np.ascontiguousarray(
        w_lin.T.reshape(2, 128, 256)).astype(ml_dtypes.bfloat16)
    bqk = np.ascontiguousarray(b_qkv[:2 * DOUT].reshape(4, 128).T)
    b_v = b_qkv[2 * DOUT:]  # unscaled: only the q section was scaled above
    bfin = (w_lin @ b_v + b_lin).astype(np.float32)
    return wqk, wv, wl, bqk, bfin


def _run(values_padded, mask, w_arrs, trace=False):
    """values_padded: [N, 256] in track-padded order; mask: None or [N]."""
    wqk, wv, wl, bqk, bfin = w_arrs
    masked = mask is not None
    nc = _get_program(masked)

    in_maps = []
    for c in range(N_CORES):
        sl = slice(c * PC, (c + 1) * PC)
        xTc = np.ascontiguousarray(
            values_padded[sl].T.reshape(2, 128, PC)
        )
        m = dict(xT=xTc, wqk=wqk, wv=wv, wl=wl, bqk=bqk, bfin=bfin)
        if masked:
            mc_ = np.ascontiguousarray(mask[sl], np.float32)
            m["mkf"] = mc_
            m["mkp"] = np.ascontiguousarray(mc_.reshape(PC // 128, 128).T)
        in_maps.append(m)

    if trace:
        _ensure_ntff_hook()
    res = run_bass_kernel_spmd(nc, in_maps, list(range(N_CORES)), trace=trace)
    outp = np.concatenate([res.results[c]["out"] for c in range(N_CORES)], 0)
    return outp, res


LAST_RESULTS = None


def kernel(values, w_qkv, b_qkv, w_lin, b_lin, track_ids, n_tracks,
           num_heads, _trace=False):
    global LAST_RESULTS
    values = np.asarray(values, np.float32)
    track_ids = np.asarray(track_ids, np.int32)
    n_tracks_i = int(n_tracks)
    num_heads_i = int(num_heads)
    assert values.shape == (N, DIN) and n_tracks_i == T and num_heads_i == H, (
        "kernel compiled for N=65536, d=256, T=256, H=8"
    )

    w_arrs = _prep_host(values, w_qkv, b_qkv, w_lin, b_lin)

    counts = np.bincount(track_ids, minlength=T)
    equal = bool((counts == L).all())

    if equal:
        outp, res = _run(values, None, w_arrs, trace=_trace)
        LAST_RESULTS = res
        return outp

    # general sorted-ragged path: scatter to padded [T, L] grid on host,
    # run the same device kernel with padding masked out of K and V, then
    # gather back (mirroring jax's oob-drop scatter / clip gather).
    starts = np.concatenate([[0], np.cumsum(counts)[:-1]])
    pos = np.arange(N, dtype=np.int64) - starts[track_ids]
    keep = pos < L
    rows = track_ids.astype(np.int64) * L + np.minimum(pos, L - 1)
    padded = np.zeros((T * L, DIN), np.float32)
    padded[rows[keep]] = values[keep]
    mask = np.zeros(T * L, np.float32)
    mask[rows[keep]] = 1.0
    outp, res = _run(padded, mask, w_arrs, trace=_trace)
    LAST_RESULTS = res
    return np.ascontiguousarray(outp[rows])
